# revision 20
# baseline (speedup 1.0000x reference)
"""Trainium2 Bass kernel for nn_AttnGlobal (B=8, N=4096, DIM=128).

reference:
    kv = x @ Wkv + bkv ; k, v = split(kv)
    q = q_global / sqrt(d)
    scores = einsum("bnd,bmd->bnm", k, q)       # softmax over m
    attn = softmax(scores, axis=-1)
    out = einsum("bnm,bmd->bnd", attn, v) @ Wp + bp

Sharding: pure data-parallel over B across the 8 cores (one batch each).

Host-side algebra folds:
    w   = x @ (Wv @ Wp)            (since attn @ (x@Wv) @ Wp = attn @ (x@(Wv@Wp)))
    bpe = bv @ Wp + bp             (since rows of attn sum to 1)

Per-core dataflow:
    xT, qT  : fp16 PE transposes                   [d, n] / [d, m]
    kT      = Wk.T @ xT + bk                       [d, n]   fp16
    S.T     = qT.T-chunks @ kT                     [m, n] tiles in PSUM (fp32)
    E.T     = exp(S.T / sqrt(d))                   fp16, ACT straight from PSUM
    U_aug   = E @ [w | 1]                          [n, 129] accumulated in PSUM
    out     = U[:, :128] * (1 / U[:, 128]) + bpe   DVE, then DMA out
"""

import os
import sys

try:
    import concourse  # noqa: F401  (resolvable via PYTHONPATH on axon images)
except ImportError:
    for _p in ("/opt/trn_rl_repo", os.path.expanduser("~/.axon_site/_ro/trn_rl_repo")):
        if os.path.isdir(_p) and _p not in sys.path:
            sys.path.append(_p)

import numpy as np

import concourse.bacc as bacc
import concourse.mybir as mybir
from concourse.bass_utils import run_bass_kernel_spmd
from concourse.tile import TileContext

B, N, D = 8, 4096, 128
NT = N // 128          # 32 row tiles
NC = N // 512          # 8 column chunks
F32 = mybir.dt.float32
F16 = mybir.dt.float16
EXP_SCALE = 1.0 / float(np.sqrt(D))

# alternating PSUM score-group sizes; sum == NT, st4 uses 4 banks, st2 uses 2
S_GROUPS = [2, 4, 2, 4, 2, 4, 2, 4, 2, 4, 2]
assert sum(S_GROUPS) == NT


def build(reps: int = 1):
    """Build and compile the per-core Bass program (identical on all cores)."""
    nc = bacc.Bacc("TRN2", target_bir_lowering=False)

    xt = nc.dram_tensor("xt", [D, N], F16, kind="ExternalInput")
    qt = nc.dram_tensor("qt", [D, N], F16, kind="ExternalInput")
    wk = nc.dram_tensor("wk", [D, D], F16, kind="ExternalInput")
    wvp = nc.dram_tensor("wvp", [D, D], F16, kind="ExternalInput")
    bk = nc.dram_tensor("bk", [D, 1], F32, kind="ExternalInput")
    bpe = nc.dram_tensor("bpe", [D, D], F32, kind="ExternalInput")  # row-tiled bias
    out = nc.dram_tensor("out", [N, D], F32, kind="ExternalOutput")

    with TileContext(nc) as tc:
        xTc = [nc.alloc_sbuf_tensor(f"xT{c}", [128, 512], F16) for c in range(NC)]
        qTc = [nc.alloc_sbuf_tensor(f"qT{c}", [128, 512], F16) for c in range(NC)]
        kTc = [nc.alloc_sbuf_tensor(f"kT{c}", [128, 512], F16) for c in range(NC)]
        w_aug = nc.alloc_sbuf_tensor("w_aug", [128, NT, 130], F16)
        ET = [
            nc.alloc_sbuf_tensor(f"et{i}", [128, NT, 512], F16) for i in range(3)
        ]
        wk_sb = nc.alloc_sbuf_tensor("wk_sb", [128, 128], F16)
        wvp_sb = nc.alloc_sbuf_tensor("wvp_sb", [128, 128], F16)
        bk_sb = nc.alloc_sbuf_tensor("bk_sb", [128, 1], F32)
        bpe_sb = nc.alloc_sbuf_tensor("bpe_sb", [128, 128], F32)

        nc.sync.dma_start(wk_sb[:], wk[:])
        nc.sync.dma_start(bk_sb[:], bk[:])
        nc.sync.dma_start(wvp_sb[:], wvp[:])

        with (
            tc.tile_pool(name="outp", bufs=4) as outp,
            tc.tile_pool(name="small", bufs=4) as small,
            tc.tile_pool(name="ps", bufs=2, space="PSUM") as psh,
            tc.tile_pool(name="st4", bufs=1, space="PSUM") as st4,
            tc.tile_pool(name="st2", bufs=1, space="PSUM") as st2,
        ):

            def s_group(c, mt, g):
                """scores S.T [m-tiles mt..mt+g, n-chunk c] -> exp -> E.T"""
                pool = st4 if g == 4 else st2
                stp = pool.tile([128, g * 512], F32, tag=f"st{g}")
                for i in range(g):
                    m = mt + i
                    nc.tensor.matmul(
                        stp[:, i * 512:(i + 1) * 512],
                        qTc[m // 4][:, (m % 4) * 128:(m % 4 + 1) * 128],
                        kTc[c][:],
                    )
                nc.scalar.activation(
                    ET[c % 3][:, mt:mt + g, :],
                    stp[:],
                    mybir.ActivationFunctionType.Exp,
                    scale=EXP_SCALE,
                )

            def s_phase(c):
                mt = 0
                for g in S_GROUPS:
                    s_group(c, mt, g)
                    mt += g

            def u_phase(c):
                buf = ET[c % 3]
                for j in range(4):
                    up = psh.tile([128, 512], F32, tag="ps")
                    for t in range(NT):
                        nc.tensor.matmul(
                            up[:, :129],
                            buf[:, t, j * 128:(j + 1) * 128],
                            w_aug[:, t, :129],
                            start=(t == 0),
                            stop=(t == NT - 1),
                        )
                    rec = small.tile([128, 1], F32, tag="rec")
                    nc.vector.reciprocal(rec[:], up[:, 128:129])
                    ot = outp.tile([128, 128], F32, tag="ot")
                    nc.vector.scalar_tensor_tensor(
                        ot[:],
                        up[:, :128],
                        rec[:],
                        bpe_sb[:],
                        mybir.AluOpType.mult,
                        mybir.AluOpType.add,
                    )
                    row = c * 512 + j * 128
                    nc.sync.dma_start(out[row:row + 128, :], ot[:])

            def body(_iv=None):
                # phase 1: host-pretransposed xT/qT chunks stream in on the two
                # HWDGE queues; kT + w_aug per chunk; S(0) rides right behind.
                nc.vector.memset(w_aug[:, :, 128:129], 1.0)
                sg = 0
                mt_done = 0
                for c in range(NC):
                    nc.sync.dma_start(xTc[c][:], xt[:, c * 512:(c + 1) * 512])
                    nc.scalar.dma_start(qTc[c][:], qt[:, c * 512:(c + 1) * 512])
                    kt = psh.tile([128, 512], F32, tag="ps")
                    nc.tensor.matmul(kt[:], wk_sb[:], xTc[c][:])
                    nc.vector.tensor_scalar_add(kTc[c][:], kt[:], bk_sb[:])
                    if c == 0:
                        nc.sync.dma_start(bpe_sb[:], bpe[:])
                    while sg < len(S_GROUPS) and mt_done + S_GROUPS[sg] <= (c + 1) * 4:
                        s_group(0, mt_done, S_GROUPS[sg])
                        mt_done += S_GROUPS[sg]
                        sg += 1
                    for i in range(4):
                        t = c * 4 + i
                        wp = psh.tile([128, 512], F32, tag="ps")
                        nc.tensor.matmul(
                            wp[:, :128],
                            xTc[c][:, i * 128:(i + 1) * 128],
                            wvp_sb[:],
                        )
                        nc.vector.tensor_copy(w_aug[:, t, :128], wp[:, :128])
                for c in range(NC):
                    if c + 1 < NC:
                        s_phase(c + 1)
                    u_phase(c)

            if reps == 1:
                body()
            else:
                with tc.For_i(0, reps, 1):
                    body()

    nc.compile()
    return nc


def _prep_weights(Wkv, bkv, Wp, bp):
    Wkv = np.asarray(Wkv, np.float32)
    bkv = np.asarray(bkv, np.float32)
    Wp = np.asarray(Wp, np.float32)
    bp = np.asarray(bp, np.float32)
    wk = np.ascontiguousarray(Wkv[:, :D].astype(np.float16))
    bk = np.ascontiguousarray(bkv[:D]).reshape(D, 1)
    wvp = np.ascontiguousarray((Wkv[:, D:] @ Wp).astype(np.float16))
    bpe_row = bkv[D:] @ Wp + bp
    bpe = np.ascontiguousarray(np.tile(bpe_row[None, :], (D, 1)))
    return wk, bk, wvp, bpe


_NC_CACHE = {}


def kernel(x, q_global, Wkv, bkv, Wp, bp):
    xt = np.asarray(x, np.float32).astype(np.float16).transpose(0, 2, 1)
    qt = np.asarray(q_global, np.float32).astype(np.float16).transpose(0, 2, 1)
    wk, bk, wvp, bpe = _prep_weights(Wkv, bkv, Wp, bp)

    if 1 not in _NC_CACHE:
        _NC_CACHE[1] = build(reps=1)
    nc = _NC_CACHE[1]

    in_maps = [
        {
            "xt": np.ascontiguousarray(xt[b]),
            "qt": np.ascontiguousarray(qt[b]),
            "wk": wk,
            "wvp": wvp,
            "bk": bk,
            "bpe": bpe,
        }
        for b in range(B)
    ]
    res = run_bass_kernel_spmd(nc, in_maps, core_ids=list(range(B)))
    return np.stack([res.results[b]["out"] for b in range(B)], axis=0)

